# revision 2
# baseline (speedup 1.0000x reference)
"""Trainium2 Bass kernel for nn_BasisDecentralizedModel (P4-equivariant GNN).

Data-parallel over the fused bs*agents=128 conv batch: 16 images per core
on 8 NeuronCores. Device computes conv1(8x8,3->32) -> relu -> conv2(5x5,
32->64) -> global-max (relu/bias folded out by monotonicity). The tiny
4-agent graph coordinator (<0.02% of FLOPs) runs vectorized on host.

Convs are lowered to shifted matmuls: the input image is replicated into
row/col-shifted partition groups so each matmul contracts (taps x chans)
with a plain free-dim offset, accumulating aligned into PSUM (no fixups).
"""

import os
import sys
import types
import contextlib
import ctypes

import numpy as np

G = 4
AGENTS = 4
BS = 32
N_CORES = 8
IMG_PER_CORE = (BS * AGENTS) // N_CORES  # 16

H = 100
W1K = 8          # conv1 kernel
C1O = 32         # conv1 out channels (8 filters x 4 rotations)
X1W = 9504       # conv1 output buffer width (100-grid rows 0..92 + margin)
C2K = 5          # conv2 kernel
C2O = 64         # conv2 out channels (16 x 4 rotations)
N_T1 = 19        # conv1 psum tiles of 500 (p in [0, 9500))
N_T2 = 18        # conv2 psum tiles of 500 (p in [0, 9000))
TN = 500         # matmul free size (<=512 fp32 psum bank)

_CACHE = {}


def _install_ntff_shim():
    """Make run_bass_kernel_spmd(trace=...) safe if BASS_TRACE is set."""
    if "antenv.axon_hooks" in sys.modules:
        return
    so_path = "/opt/axon/libaxon_pjrt.so"

    def _make_hook():
        try:
            lib = ctypes.CDLL(so_path)
        except OSError:
            return None
        if not hasattr(lib, "axon_start_nrt_profile"):
            return None
        lib.axon_start_nrt_profile.argtypes = [ctypes.POINTER(ctypes.c_int64),
                                               ctypes.c_size_t]
        lib.axon_start_nrt_profile.restype = ctypes.c_int64
        lib.axon_stop_nrt_profile.argtypes = [ctypes.c_char_p]
        lib.axon_stop_nrt_profile.restype = ctypes.c_int64

        @contextlib.contextmanager
        def _hook(output_dir, device_ids):
            import jax
            jax.devices()
            if device_ids:
                ids = (ctypes.c_int64 * len(device_ids))(*device_ids)
                rc = lib.axon_start_nrt_profile(ids, len(device_ids))
            else:
                rc = lib.axon_start_nrt_profile(None, 0)
            if rc != 0:
                raise RuntimeError(f"axon_start_nrt_profile rc={rc}")
            try:
                yield
            finally:
                n = lib.axon_stop_nrt_profile(str(output_dir).encode())
                print(f"profile: {n} file(s) in {output_dir}", file=sys.stderr)

        return _hook

    mod = types.ModuleType("antenv.axon_hooks")
    hook = _make_hook()
    mod.get_axon_ntff_profile_hook = lambda: hook
    mod.set_axon_ntff_profile_hook = lambda h: None
    sys.modules["antenv.axon_hooks"] = mod


def _build_nc():
    import concourse.bacc as bacc
    import concourse.mybir as mybir
    globals()['mybir'] = mybir
    from concourse.tile import TileContext

    f32 = mybir.dt.float32
    f32r = mybir.dt.float32r

    nc = bacc.Bacc("TRN2", target_bir_lowering=False, debug=False,
                   num_devices=N_CORES)

    img_d = nc.dram_tensor("img", [IMG_PER_CORE, 3, H * H], f32r,
                           kind="ExternalInput").ap()
    w1_d = nc.dram_tensor("w1t", [2, 96, C1O], f32r, kind="ExternalInput").ap()
    w2a_d = nc.dram_tensor("w2a", [C2K, 128, C2O], f32r,
                           kind="ExternalInput").ap()
    w2b_d = nc.dram_tensor("w2b", [C2K, 32, C2O], f32r,
                           kind="ExternalInput").ap()
    b1_d = nc.dram_tensor("b1v", [32, 1], f32, kind="ExternalInput").ap()
    feat_d = nc.dram_tensor("feat", [C2O, IMG_PER_CORE], f32,
                            kind="ExternalOutput").ap()

    with TileContext(nc) as tc:
        with (
            tc.tile_pool(name="wpool", bufs=1) as wpool,
            tc.tile_pool(name="r1pool", bufs=2) as r1pool,
            tc.tile_pool(name="r2pool", bufs=2) as r2pool,
            tc.tile_pool(name="mxpool", bufs=3) as mxpool,
            tc.tile_pool(name="psum1", bufs=4, space="PSUM") as ps1pool,
            tc.tile_pool(name="psum2", bufs=4, space="PSUM") as ps2pool,
        ):
            w1_sb = wpool.tile([96, 2 * C1O], f32r, tag="w1")
            nc.sync.dma_start(out=w1_sb[:, 0:C1O], in_=w1_d[0])
            nc.sync.dma_start(out=w1_sb[:, C1O:2 * C1O], in_=w1_d[1])
            w2a_sb = wpool.tile([128, C2K * C2O], f32r, tag="w2a")
            w2b_sb = wpool.tile([32, C2K * C2O], f32r, tag="w2b")
            for dx in range(C2K):
                nc.sync.dma_start(out=w2a_sb[:, dx * C2O:(dx + 1) * C2O],
                                  in_=w2a_d[dx])
                nc.sync.dma_start(out=w2b_sb[:, dx * C2O:(dx + 1) * C2O],
                                  in_=w2b_d[dx])
            b1_sb = wpool.tile([32, 1], f32, tag="b1")
            nc.sync.dma_start(out=b1_sb[:], in_=b1_d[:])

            feat_sb = wpool.tile([C2O, IMG_PER_CORE], f32, tag="feat")

            for i in range(IMG_PER_CORE):
                # R1: 32 shifted copies (dy 0..7 x dxb {0,2,4,6}) x 3 ch.
                r1 = r1pool.tile([96, X1W], f32r, tag="r1")
                for dy in range(8):
                    for db2 in range(4):
                        g = dy * 4 + db2
                        off = dy * H + 2 * db2
                        ln = min(X1W, H * H - off)
                        nc.sync.dma_start(out=r1[3 * g:3 * g + 3, 0:ln],
                                          in_=img_d[i, :, off:off + ln])

                # R2: conv1 output (group 0) + 3 row-shifted copies.
                r2 = r2pool.tile([128, X1W], f32r, tag="r2")

                for k in range(N_T1):
                    ps = ps1pool.tile([C1O, TN], f32, tag="p1")
                    for t in range(2):
                        nc.tensor.matmul(
                            ps[:],
                            lhsT=w1_sb[:, t * C1O:(t + 1) * C1O],
                            rhs=r1[:, k * TN + t:k * TN + t + TN],
                            start=(t == 0), stop=(t == 1),
                        )
                    # relu(x + b1) into R2 group 0
                    nc.vector.tensor_scalar(
                        out=r2[0:C1O, k * TN:(k + 1) * TN],
                        in0=ps[:], scalar1=b1_sb[:], scalar2=0.0,
                        op0=mybir.AluOpType.add,
                        op1=mybir.AluOpType.max,
                    )

                for r in range(1, 4):
                    nc.sync.dma_start(
                        out=r2[32 * r:32 * r + 32, 0:X1W - H * r],
                        in_=r2[0:32, H * r:X1W],
                    )

                mx = mxpool.tile([C2O, N_T2], f32, tag="mx")
                for k in range(N_T2):
                    ps = ps2pool.tile([C2O, TN], f32, tag="p2")
                    for dx in range(C2K):
                        nc.tensor.matmul(
                            ps[:],
                            lhsT=w2a_sb[:, dx * C2O:(dx + 1) * C2O],
                            rhs=r2[:, k * TN + dx:k * TN + dx + TN],
                            start=(dx == 0), stop=False,
                        )
                    for dx in range(C2K):
                        nc.tensor.matmul(
                            ps[:],
                            lhsT=w2b_sb[:, dx * C2O:(dx + 1) * C2O],
                            rhs=r2[0:32, k * TN + 400 + dx:k * TN + 400 + dx + TN],
                            start=False, stop=(dx == C2K - 1),
                        )
                    rows_v = min(5, 89 - 5 * k)
                    red_in = (ps[:, 0:rows_v * H]
                              .rearrange("p (r w) -> p r w", w=H)[:, :, 0:89])
                    nc.vector.reduce_max(mx[:, k:k + 1], red_in,
                                         axis=mybir.AxisListType.XY)

                nc.vector.reduce_max(feat_sb[:, i:i + 1], mx[:],
                                     axis=mybir.AxisListType.X)

            nc.sync.dma_start(out=feat_d[:], in_=feat_sb[:])

    nc.compile()
    return nc


def _prep_weights(W1, W2):
    K1 = np.stack([np.rot90(W1, k=r, axes=(2, 3)) for r in range(G)],
                  axis=1).reshape(C1O, 3, W1K, W1K)
    rot = np.stack([np.rot90(W2, k=r, axes=(-2, -1)) for r in range(G)], axis=0)
    idx = np.arange(G)
    K2 = np.stack([rot[r][:, :, (idx - r) % G] for r in range(G)],
                  axis=1).reshape(C2O, 32, C2K, C2K)

    # conv1 lhsT: [t, p=(dy*4+db2)*3+c, o] = K1[o, c, dy, 2*db2 + t]
    w1t = np.zeros((2, 96, C1O), np.float32)
    for t in range(2):
        for dy in range(8):
            for db2 in range(4):
                g = dy * 4 + db2
                # [o, c] -> [c, o]
                w1t[t, 3 * g:3 * g + 3, :] = K1[:, :, dy, 2 * db2 + t].T

    # conv2 lhsT A: [dx, p=32*dy+c', o'] = K2[o', c', dy, dx] (dy 0..3)
    w2a = np.zeros((C2K, 128, C2O), np.float32)
    for dx in range(C2K):
        for dy in range(4):
            w2a[dx, 32 * dy:32 * dy + 32, :] = K2[:, :, dy, dx].T
    # conv2 lhsT B: dy=4 row on R2 group 0 (offset +400)
    w2b = np.zeros((C2K, 32, C2O), np.float32)
    for dx in range(C2K):
        w2b[dx] = K2[:, :, 4, dx].T
    return w1t, w2a, w2b


def kernel(diffs, states, W1, b1, W2, b2, Wg, bg, W4, b4, W5, b5):
    _install_ntff_shim()
    from concourse.bass_utils import run_bass_kernel_spmd

    diffs = np.asarray(diffs, np.float32)
    states = np.asarray(states, np.float32)
    W1 = np.asarray(W1, np.float32)
    W2 = np.asarray(W2, np.float32)

    if "nc" not in _CACHE:
        _CACHE["nc"] = _build_nc()
    nc = _CACHE["nc"]

    w1t, w2a, w2b = _prep_weights(W1, W2)
    b1v = np.repeat(np.asarray(b1, np.float32), G)[:, None].copy()  # [32,1]

    x = states.reshape(BS * AGENTS, 3, H * H)
    in_maps = []
    for c in range(N_CORES):
        in_maps.append({
            "img": np.ascontiguousarray(
                x[c * IMG_PER_CORE:(c + 1) * IMG_PER_CORE]),
            "w1t": w1t, "w2a": w2a, "w2b": w2b, "b1v": b1v,
        })

    res = run_bass_kernel_spmd(nc, in_maps, list(range(N_CORES)),
                               trace=bool(os.environ.get("BASS_TRACE")))
    _CACHE["last_exec_time_ns"] = res.exec_time_ns

    feat = np.concatenate([res.results[c]["feat"].T for c in range(N_CORES)],
                          axis=0)                       # [128, 64]
    feat = feat.reshape(BS, AGENTS, 16, G)

    # ---- host epilogue (tiny graph coordinator) ----
    b2 = np.asarray(b2, np.float32)
    feat = np.maximum(feat + b2[None, None, :, None], 0.0)

    locs = diffs
    dvec = locs[:, :, None, :] - locs[:, None, :, :]
    norms = np.sqrt(np.sum(dvec * dvec, axis=-1) + 1e-12)
    agg = np.einsum("bij,bjcg->bicg", norms, feat)
    z = np.concatenate([feat, agg], axis=2)             # [bs, a, 32, g]

    idx = (np.arange(G)[:, None] - np.arange(G)[None, :]) % G
    Wgh = np.asarray(Wg, np.float32)[:, :, idx]         # [o, c, h, g]
    h_st = np.maximum(
        np.einsum("bicg,ochg->bioh", z, Wgh)
        + np.asarray(bg, np.float32)[None, None, :, None], 0.0)

    N_ACT = 2
    a_idx = (np.arange(G)[:, None] - np.arange(N_ACT)[None, :]) % N_ACT
    W4g = np.asarray(W4, np.float32)[:, a_idx]          # [c, g, a]
    policies = np.einsum("bicg,cga->bia", h_st, W4g) + np.asarray(b4, np.float32)
    values = np.einsum("bicg,c->bi", h_st, np.asarray(W5, np.float32)) \
        + np.asarray(b5, np.float32)
    return policies.astype(np.float32), values.astype(np.float32)


# revision 3
# speedup vs baseline: 1.1301x; 1.1301x over previous
"""Trainium2 Bass kernel for nn_BasisDecentralizedModel (P4-equivariant GNN).

Data-parallel over the fused bs*agents=128 conv batch: 16 images per core
on 8 NeuronCores. Device computes conv1(8x8,3->32) -> relu -> conv2(5x5,
32->64) -> global-max (relu/bias folded out by monotonicity). The tiny
4-agent graph coordinator (<0.02% of FLOPs) runs vectorized on host.

Convs are lowered to shifted matmuls: the input image is replicated into
row/col-shifted partition groups so each matmul contracts (taps x chans)
with a plain free-dim offset, accumulating aligned into PSUM (no fixups).
"""

import os
import sys
import types
import contextlib
import ctypes

import numpy as np

G = 4
AGENTS = 4
BS = 32
N_CORES = 8
IMG_PER_CORE = (BS * AGENTS) // N_CORES  # 16

H = 100
W1K = 8          # conv1 kernel
C1O = 32         # conv1 out channels (8 filters x 4 rotations)
X1W = 9504       # conv1 output buffer width (100-grid rows 0..92 + margin)
C2K = 5          # conv2 kernel
C2O = 64         # conv2 out channels (16 x 4 rotations)
N_T1 = 19        # conv1 psum tiles of 500 (p in [0, 9500))
N_T2 = 18        # conv2 psum tiles of 500 (p in [0, 9000))
TN = 500         # matmul free size (<=512 fp32 psum bank)

_CACHE = {}


def _install_ntff_shim():
    """Make run_bass_kernel_spmd(trace=...) safe if BASS_TRACE is set."""
    if "antenv.axon_hooks" in sys.modules:
        return
    so_path = "/opt/axon/libaxon_pjrt.so"

    def _make_hook():
        try:
            lib = ctypes.CDLL(so_path)
        except OSError:
            return None
        if not hasattr(lib, "axon_start_nrt_profile"):
            return None
        lib.axon_start_nrt_profile.argtypes = [ctypes.POINTER(ctypes.c_int64),
                                               ctypes.c_size_t]
        lib.axon_start_nrt_profile.restype = ctypes.c_int64
        lib.axon_stop_nrt_profile.argtypes = [ctypes.c_char_p]
        lib.axon_stop_nrt_profile.restype = ctypes.c_int64

        @contextlib.contextmanager
        def _hook(output_dir, device_ids):
            import jax
            jax.devices()
            if device_ids:
                ids = (ctypes.c_int64 * len(device_ids))(*device_ids)
                rc = lib.axon_start_nrt_profile(ids, len(device_ids))
            else:
                rc = lib.axon_start_nrt_profile(None, 0)
            if rc != 0:
                raise RuntimeError(f"axon_start_nrt_profile rc={rc}")
            try:
                yield
            finally:
                n = lib.axon_stop_nrt_profile(str(output_dir).encode())
                print(f"profile: {n} file(s) in {output_dir}", file=sys.stderr)

        return _hook

    mod = types.ModuleType("antenv.axon_hooks")
    hook = _make_hook()
    mod.get_axon_ntff_profile_hook = lambda: hook
    mod.set_axon_ntff_profile_hook = lambda h: None
    sys.modules["antenv.axon_hooks"] = mod


def _build_nc():
    import concourse.bacc as bacc
    import concourse.mybir as mybir
    globals()['mybir'] = mybir
    from concourse.tile import TileContext

    f32 = mybir.dt.float32
    f32r = mybir.dt.bfloat16  # matmul operand dtype (psum accum stays fp32)

    nc = bacc.Bacc("TRN2", target_bir_lowering=False, debug=False,
                   num_devices=N_CORES)

    img_d = nc.dram_tensor("img", [IMG_PER_CORE, 3, H * H], f32r,
                           kind="ExternalInput").ap()
    w1_d = nc.dram_tensor("w1t", [2, 96, C1O], f32r, kind="ExternalInput").ap()
    w2a_d = nc.dram_tensor("w2a", [C2K, 128, C2O], f32r,
                           kind="ExternalInput").ap()
    w2b_d = nc.dram_tensor("w2b", [C2K, 32, C2O], f32r,
                           kind="ExternalInput").ap()
    b1_d = nc.dram_tensor("b1v", [32, 1], f32, kind="ExternalInput").ap()
    feat_d = nc.dram_tensor("feat", [C2O, IMG_PER_CORE], f32,
                            kind="ExternalOutput").ap()

    with TileContext(nc) as tc:
        with (
            tc.tile_pool(name="wpool", bufs=1) as wpool,
            tc.tile_pool(name="r1pool", bufs=2) as r1pool,
            tc.tile_pool(name="r2pool", bufs=2) as r2pool,
            tc.tile_pool(name="mxpool", bufs=3) as mxpool,
            tc.tile_pool(name="psum1", bufs=4, space="PSUM") as ps1pool,
            tc.tile_pool(name="psum2", bufs=4, space="PSUM") as ps2pool,
        ):
            w1_sb = wpool.tile([96, 2 * C1O], f32r, tag="w1")
            nc.sync.dma_start(out=w1_sb[:, 0:C1O], in_=w1_d[0])
            nc.sync.dma_start(out=w1_sb[:, C1O:2 * C1O], in_=w1_d[1])
            w2a_sb = wpool.tile([128, C2K * C2O], f32r, tag="w2a")
            w2b_sb = wpool.tile([32, C2K * C2O], f32r, tag="w2b")
            for dx in range(C2K):
                nc.sync.dma_start(out=w2a_sb[:, dx * C2O:(dx + 1) * C2O],
                                  in_=w2a_d[dx])
                nc.sync.dma_start(out=w2b_sb[:, dx * C2O:(dx + 1) * C2O],
                                  in_=w2b_d[dx])
            b1_sb = wpool.tile([32, 1], f32, tag="b1")
            nc.sync.dma_start(out=b1_sb[:], in_=b1_d[:])

            feat_sb = wpool.tile([C2O, IMG_PER_CORE], f32, tag="feat")

            for i in range(IMG_PER_CORE):
                # R1: 32 shifted copies (dy 0..7 x dxb {0,2,4,6}) x 3 ch.
                r1 = r1pool.tile([96, X1W], f32r, tag="r1")
                for dy in range(8):
                    for db2 in range(4):
                        g = dy * 4 + db2
                        off = dy * H + 2 * db2
                        ln = min(X1W, H * H - off)
                        nc.sync.dma_start(out=r1[3 * g:3 * g + 3, 0:ln],
                                          in_=img_d[i, :, off:off + ln])

                # R2: conv1 output (group 0) + 3 row-shifted copies.
                r2 = r2pool.tile([128, X1W], f32r, tag="r2")

                for k in range(N_T1):
                    ps = ps1pool.tile([C1O, TN], f32, tag="p1")
                    for t in range(2):
                        nc.tensor.matmul(
                            ps[:],
                            lhsT=w1_sb[:, t * C1O:(t + 1) * C1O],
                            rhs=r1[:, k * TN + t:k * TN + t + TN],
                            start=(t == 0), stop=(t == 1),
                        )
                    # relu(x + b1) into R2 group 0
                    nc.vector.tensor_scalar(
                        out=r2[0:C1O, k * TN:(k + 1) * TN],
                        in0=ps[:], scalar1=b1_sb[:], scalar2=0.0,
                        op0=mybir.AluOpType.add,
                        op1=mybir.AluOpType.max,
                    )

                for r in range(1, 4):
                    nc.sync.dma_start(
                        out=r2[32 * r:32 * r + 32, 0:X1W - H * r],
                        in_=r2[0:32, H * r:X1W],
                    )

                mx = mxpool.tile([C2O, N_T2], f32, tag="mx")
                for k in range(N_T2):
                    ps = ps2pool.tile([C2O, TN], f32, tag="p2")
                    for dx in range(C2K):
                        nc.tensor.matmul(
                            ps[:],
                            lhsT=w2a_sb[:, dx * C2O:(dx + 1) * C2O],
                            rhs=r2[:, k * TN + dx:k * TN + dx + TN],
                            start=(dx == 0), stop=False,
                        )
                    for dx in range(C2K):
                        nc.tensor.matmul(
                            ps[:],
                            lhsT=w2b_sb[:, dx * C2O:(dx + 1) * C2O],
                            rhs=r2[0:32, k * TN + 400 + dx:k * TN + 400 + dx + TN],
                            start=False, stop=(dx == C2K - 1),
                        )
                    rows_v = min(5, 89 - 5 * k)
                    red_in = (ps[:, 0:rows_v * H]
                              .rearrange("p (r w) -> p r w", w=H)[:, :, 0:89])
                    nc.vector.reduce_max(mx[:, k:k + 1], red_in,
                                         axis=mybir.AxisListType.XY)

                nc.vector.reduce_max(feat_sb[:, i:i + 1], mx[:],
                                     axis=mybir.AxisListType.X)

            nc.sync.dma_start(out=feat_d[:], in_=feat_sb[:])

    nc.compile()
    return nc


def _prep_weights(W1, W2):
    K1 = np.stack([np.rot90(W1, k=r, axes=(2, 3)) for r in range(G)],
                  axis=1).reshape(C1O, 3, W1K, W1K)
    rot = np.stack([np.rot90(W2, k=r, axes=(-2, -1)) for r in range(G)], axis=0)
    idx = np.arange(G)
    K2 = np.stack([rot[r][:, :, (idx - r) % G] for r in range(G)],
                  axis=1).reshape(C2O, 32, C2K, C2K)

    # conv1 lhsT: [t, p=(dy*4+db2)*3+c, o] = K1[o, c, dy, 2*db2 + t]
    w1t = np.zeros((2, 96, C1O), np.float32)
    for t in range(2):
        for dy in range(8):
            for db2 in range(4):
                g = dy * 4 + db2
                # [o, c] -> [c, o]
                w1t[t, 3 * g:3 * g + 3, :] = K1[:, :, dy, 2 * db2 + t].T

    # conv2 lhsT A: [dx, p=32*dy+c', o'] = K2[o', c', dy, dx] (dy 0..3)
    w2a = np.zeros((C2K, 128, C2O), np.float32)
    for dx in range(C2K):
        for dy in range(4):
            w2a[dx, 32 * dy:32 * dy + 32, :] = K2[:, :, dy, dx].T
    # conv2 lhsT B: dy=4 row on R2 group 0 (offset +400)
    w2b = np.zeros((C2K, 32, C2O), np.float32)
    for dx in range(C2K):
        w2b[dx] = K2[:, :, 4, dx].T
    return w1t, w2a, w2b


def kernel(diffs, states, W1, b1, W2, b2, Wg, bg, W4, b4, W5, b5):
    _install_ntff_shim()
    from concourse.bass_utils import run_bass_kernel_spmd

    diffs = np.asarray(diffs, np.float32)
    states = np.asarray(states, np.float32)
    W1 = np.asarray(W1, np.float32)
    W2 = np.asarray(W2, np.float32)

    if "nc" not in _CACHE:
        _CACHE["nc"] = _build_nc()
    nc = _CACHE["nc"]

    w1t, w2a, w2b = _prep_weights(W1, W2)
    b1v = np.repeat(np.asarray(b1, np.float32), G)[:, None].copy()  # [32,1]

    import ml_dtypes
    bf16 = ml_dtypes.bfloat16
    w1t = w1t.astype(bf16)
    w2a = w2a.astype(bf16)
    w2b = w2b.astype(bf16)
    x = states.reshape(BS * AGENTS, 3, H * H).astype(bf16)
    in_maps = []
    for c in range(N_CORES):
        in_maps.append({
            "img": np.ascontiguousarray(
                x[c * IMG_PER_CORE:(c + 1) * IMG_PER_CORE]),
            "w1t": w1t, "w2a": w2a, "w2b": w2b, "b1v": b1v,
        })

    res = run_bass_kernel_spmd(nc, in_maps, list(range(N_CORES)),
                               trace=bool(os.environ.get("BASS_TRACE")))
    _CACHE["last_exec_time_ns"] = res.exec_time_ns

    feat = np.concatenate([res.results[c]["feat"].T for c in range(N_CORES)],
                          axis=0)                       # [128, 64]
    feat = feat.reshape(BS, AGENTS, 16, G)

    # ---- host epilogue (tiny graph coordinator) ----
    b2 = np.asarray(b2, np.float32)
    feat = np.maximum(feat + b2[None, None, :, None], 0.0)

    locs = diffs
    dvec = locs[:, :, None, :] - locs[:, None, :, :]
    norms = np.sqrt(np.sum(dvec * dvec, axis=-1) + 1e-12)
    agg = np.einsum("bij,bjcg->bicg", norms, feat)
    z = np.concatenate([feat, agg], axis=2)             # [bs, a, 32, g]

    idx = (np.arange(G)[:, None] - np.arange(G)[None, :]) % G
    Wgh = np.asarray(Wg, np.float32)[:, :, idx]         # [o, c, h, g]
    h_st = np.maximum(
        np.einsum("bicg,ochg->bioh", z, Wgh)
        + np.asarray(bg, np.float32)[None, None, :, None], 0.0)

    N_ACT = 2
    a_idx = (np.arange(G)[:, None] - np.arange(N_ACT)[None, :]) % N_ACT
    W4g = np.asarray(W4, np.float32)[:, a_idx]          # [c, g, a]
    policies = np.einsum("bicg,cga->bia", h_st, W4g) + np.asarray(b4, np.float32)
    values = np.einsum("bicg,c->bi", h_st, np.asarray(W5, np.float32)) \
        + np.asarray(b5, np.float32)
    return policies.astype(np.float32), values.astype(np.float32)
